# revision 45
# baseline (speedup 1.0000x reference)
"""Multi-head self-attention (B=2, S=2048, D=1024, H=16, causal) on 8 NeuronCores.

Sharding: core c = 4*b + g handles batch b and heads 4g..4g+3 (batch x
head-group parallel). Per core:
  - q/k projections in transposed layout qT/kT [dh, s] (dh on partitions,
    two heads stacked per 128-partition tile), evicted to bf16
  - v projection in natural layout [s, dh], bf16, with a fused ones-column
    per head (softmax denominator comes free during the AV matmul); tiles
    are pre-memset to 1.0 so the ones columns need no strided write
  - attention per HEAD PAIR in scoresT [j, i] orientation: bf16 score
    matmuls for the two heads at row groups 0:64 / 64:128, causal masking
    as an additive -704 bias on the PSUM scores BEFORE the exp (so exp
    underflows to zero; avoids in-place bf16 multiplies, which corrupt),
    exp sliced to live regions on diagonal chunks, bf16 probs, AV matmuls
    column-sliced to the causal live region
  - softmax normalization per pair: dens to f32r rows 0/32 of a shared
    tile, one K=33 selector matmul broadcasts both dens across partitions,
    one [128,512] reciprocal, two multiplies into mergedT
  - o-projection partials staged via SBUF and DMA'd out; all projection
    work is sliced into ~0.5-2us quanta drained inside the attention
    j-loop to fill the PE's exp-wait gaps
Host sums the 4 partial outputs per batch (the only cross-core reduction).

Projections run in f32r (weights with 128 columns in bf16 would take the
compiler's FWL path, which corrupts weight loads in this kernel); the
attention matmuls (K=64 scores / M=65 AV, both FWL-ineligible) run in bf16.
"""

import numpy as np
import ml_dtypes
from collections import deque

import concourse.bass as bass
from concourse import bacc
import concourse.mybir as mybir
import concourse.tile as tile
from concourse import bass_utils

F32 = mybir.dt.float32
F32R = mybir.dt.float32r
BF16 = mybir.dt.bfloat16
EXP = mybir.ActivationFunctionType.Exp

B, S, D = 2, 2048, 1024
H, DH = 16, 64
NCORES = 8
HPG = 4                  # heads per group (per core)
M = HPG * DH             # 256 per-core head dims
VW = DH + 2              # v tile cols per head: 64 v + 1 ones + 1 pad (even stride)
DC = D // 128            # 8 contraction chunks for projections
IC = 512                 # i (query) chunk for attention
JC = 128                 # j (key) chunk for attention
SCALE = 1.0 / np.sqrt(DH)
import os
PAIR_ALIGN = os.environ.get("PAIR_ALIGN", "0") == "1"
LOADBF = os.environ.get("LOADBF", "0") == "1"
XW16 = os.environ.get("XW16", "0") == "1"   # x/w/wo loads bf16
QK16 = os.environ.get("QK16", "1") == "1"   # q/k tiles bf16
AT16 = os.environ.get("AT16", "1") == "1"   # v/probs/tri bf16
MG16 = os.environ.get("MG16", "0") == "1"   # mg bf16 (wo must match)
XWDT = BF16 if XW16 else F32R
QKDT = BF16 if QK16 else F32R
ATDT = BF16 if AT16 else F32R
ATMT = BF16 if AT16 else F32
MGDT = BF16 if MG16 else F32R


def _build_nc():
    nc = bacc.Bacc("TRN2", target_bir_lowering=False, debug=False)

    xT_d = nc.dram_tensor("xT", [D, S // 2] if LOADBF else [D, S], F32R, kind="ExternalInput").ap()
    wqkv_d = nc.dram_tensor("wqkvT", [D, 3 * M // 2] if LOADBF else [D, 3 * M], F32R, kind="ExternalInput").ap()
    woT_d = nc.dram_tensor("woT", [M, D], MGDT, kind="ExternalInput").ap()
    tri_d = nc.dram_tensor("tri", [JC, JC], F32, kind="ExternalInput").ap()
    sel_d = nc.dram_tensor("sel", [33, 128], F32R, kind="ExternalInput").ap()
    onesb_d = nc.dram_tensor("ones_b", [JC, HPG], ATDT, kind="ExternalInput").ap()
    out_d = nc.dram_tensor("out", [S, D], BF16, kind="ExternalOutput").ap()
    dbg = {}
    if os.environ.get("KDBG") == "1":
        dbg["q0"] = nc.dram_tensor("dbg_q0", [128, S], QKDT, kind="ExternalOutput").ap()
        dbg["k0"] = nc.dram_tensor("dbg_k0", [128, S], QKDT, kind="ExternalOutput").ap()
        dbg["v0"] = nc.dram_tensor("dbg_v0", [JC, HPG * VW], ATDT, kind="ExternalOutput").ap()
        dbg["mg0"] = nc.dram_tensor("dbg_mg0", [128, S], MGDT, kind="ExternalOutput").ap()
        dbg["pr0"] = nc.dram_tensor("dbg_pr0", [JC, IC], ATDT, kind="ExternalOutput").ap()
        dbg["x0"] = nc.dram_tensor("dbg_x0", [128, S], F32, kind="ExternalOutput").ap()
        dbg["w0"] = nc.dram_tensor("dbg_w0", [128, 3 * M], F32, kind="ExternalOutput").ap()

    with tile.TileContext(nc) as tc:
        _body(tc, xT_d, wqkv_d, woT_d, tri_d, sel_d, onesb_d, out_d, dbg)
    nc.compile()
    return nc


def _body(tc, xT_d, wqkv_d, woT_d, tri_d, sel_d, onesb_d, out_d, dbg=None):
    nc = tc.nc
    from contextlib import ExitStack
    ctx = ExitStack()
    with ctx:
        p_x = ctx.enter_context(tc.tile_pool(name="x", bufs=DC))
        p_w = ctx.enter_context(tc.tile_pool(name="w", bufs=DC))
        p_xl = ctx.enter_context(tc.tile_pool(name="xl", bufs=DC))
        p_wl = ctx.enter_context(tc.tile_pool(name="wl", bufs=DC))
        p_wo = ctx.enter_context(tc.tile_pool(name="wo", bufs=2))
        p_qk = ctx.enter_context(tc.tile_pool(name="qk", bufs=2))
        p_v = ctx.enter_context(tc.tile_pool(name="v", bufs=S // JC))
        p_mg = ctx.enter_context(tc.tile_pool(name="mg", bufs=2))
        p_probs = ctx.enter_context(tc.tile_pool(name="probs", bufs=6))
        p_small = ctx.enter_context(tc.tile_pool(name="small", bufs=2))
        p_mask = ctx.enter_context(tc.tile_pool(name="mask", bufs=1))
        p_ostg = ctx.enter_context(tc.tile_pool(name="ostg", bufs=2))
        p_ones = ctx.enter_context(tc.tile_pool(name="ones", bufs=1))

        # PSUM: 8 banks total.
        ps_sc = ctx.enter_context(tc.tile_pool(name="pss", bufs=2, space="PSUM"))  # 2x[128,1024] = 4 banks
        ps_at = ctx.enter_context(tc.tile_pool(name="psa", bufs=2, space="PSUM"))  # 2x[65,512]  = 2 banks
        ps_pj = ctx.enter_context(tc.tile_pool(name="psp", bufs=2, space="PSUM"))  # 2x[128,512] = 2 banks

        # ---- HAM pre-warm: keep the PE activity monitor busy while the
        # first x/w tiles stream in so the clock gate is at full rate when
        # the real projections start.
        wrm = p_ones.tile([128, 512], F32, tag="warm")
        nc.vector.memset(wrm[:], 1.0)
        wrm_ps = ps_pj.tile([128, 512], F32, tag="proj", name="warmps")
        for r in range(9):
            nc.tensor.matmul(wrm_ps[:], wrm[:, 0:128], wrm[:],
                             start=(r == 0), stop=(r == 8))
        nc.scalar.copy(wrm[:, 0:1], wrm_ps[:, 0:1])  # keep alive vs DCE

        # ---- input loads, interleaved in consumption order. x/w stream in
        # as bf16 (halves the DMA-bound ramp) and are converted to f32r on
        # DVE while the ramp is otherwise idle (bf16 matmul weights with 128
        # cols trip the FWL path, which corrupts in this kernel - so the
        # projections consume f32r).
        w_t, x_t = [], []
        for dc in range(DC):
            if LOADBF:
                wl = p_wl.tile([128, 3 * M // 2], F32R, tag="wl")
                nc.sync.dma_start(wl[:], wqkv_d[dc * 128:(dc + 1) * 128, :])
                xl = p_xl.tile([128, S // 2], F32R, tag="xl")
                nc.sync.dma_start(xl[:], xT_d[dc * 128:(dc + 1) * 128, :])
                wt = p_w.tile([128, 3 * M], F32R, tag="w")
                nc.scalar.copy(wt[:], wl[:].bitcast(BF16))
                w_t.append(wt)
                xt = p_x.tile([128, S], F32R, tag="x")
                nc.scalar.copy(xt[:], xl[:].bitcast(BF16))
                x_t.append(xt)
            else:
                wt = p_w.tile([128, 3 * M], F32R, tag="w")
                nc.sync.dma_start(wt[:], wqkv_d[dc * 128:(dc + 1) * 128, :])
                w_t.append(wt)
                xt = p_x.tile([128, S], F32R, tag="x")
                nc.sync.dma_start(xt[:, 0:512], xT_d[dc * 128:(dc + 1) * 128, 0:512])
                x_t.append(xt)
        if not LOADBF:
            # remaining x column blocks, s4-major: the first attention wave
            # (ic=0) only needs columns 0:512, so it can start ~3x earlier
            for s4 in range(1, 4):
                for dc in range(DC):
                    nc.sync.dma_start(
                        x_t[dc][:, s4 * 512:(s4 + 1) * 512],
                        xT_d[dc * 128:(dc + 1) * 128, s4 * 512:(s4 + 1) * 512])
        wo_t = []
        for kc in range(2):
            t = p_wo.tile([128, D], MGDT, tag="wo")
            nc.sync.dma_start(t[:], woT_d[kc * 128:(kc + 1) * 128, :])
            wo_t.append(t)
        tri_t = p_mask.tile([JC, JC], F32, tag="tri")
        nc.sync.dma_start(tri_t[:], tri_d[:])
        sel_t = p_ones.tile([33, 128], F32R, tag="sel")
        nc.sync.dma_start(sel_t[:], sel_d[:])
        denr_t = p_ones.tile([33, IC], F32R, tag="denr")
        nc.vector.memset(denr_t[:].bitcast(F32), 0.0)
        onesb_t = p_ones.tile([JC, HPG], ATDT, tag="onesb")
        nc.sync.dma_start(onesb_t[:], onesb_d[:])

        # ---- projection building blocks ----
        q_t, k_t = {}, {}

        def qk_quant(mc, woff, store, tg, s4):
            # one qT/kT [128, 512] block: 8-matmul contraction chain + one
            # DVE eviction copy
            sl = slice(s4 * 512, (s4 + 1) * 512)
            dst = store.get(mc)
            if dst is None:
                dst = p_qk.tile([128, S], QKDT, tag=tg, name=f"{tg}{mc}")
                store[mc] = dst
            ps = ps_pj.tile([128, 512], F32, tag="proj")
            for dc in range(DC):
                nc.tensor.matmul(
                    ps[:],
                    w_t[dc][:, woff + mc * 128:woff + (mc + 1) * 128],
                    x_t[dc][:, sl],
                    start=(dc == 0), stop=(dc == DC - 1))
            nc.vector.tensor_copy(dst[:, sl], ps[:])

        v_pre = {}

        def v_premake(sc):
            # pre-created + memset to 1.0 (ones cols come for free; the
            # eviction overwrites the value cols)
            vt = p_v.tile([JC, HPG * VW], ATDT, tag="v", name=f"v{sc}")
            if AT16:
                nc.vector.memset(vt[:].bitcast(mybir.dt.uint16), 16256)  # bf16 1.0
            else:
                nc.vector.memset(vt[:].bitcast(F32), 1.0)
            v_pre[sc] = vt

        def v_quant(sc):
            # v[s, m] tile for j-chunk sc: per head h cols h*66..h*66+63 = v,
            # col h*66+64 = 1.0 (softmax denominator column)
            vt = v_pre[sc]
            ps = ps_pj.tile([128, 512], F32, tag="proj")
            for dc in range(DC):
                nc.tensor.matmul(
                    ps[:, 0:M],
                    x_t[dc][:, sc * 128:(sc + 1) * 128],
                    w_t[dc][:, 2 * M:3 * M],
                    start=(dc == 0), stop=(dc == DC - 1))
            nc.vector.tensor_copy(
                vt[:].rearrange("p (h e) -> p h e", h=HPG)[:, :, 0:DH],
                ps[:, 0:M].rearrange("p (h d) -> p h d", h=HPG))
            v_t[sc] = vt

        v_t = {}

        ostg_t = {}

        def oproj_half(sc, nn):
            # half of an out[s, o] partial: 2 matmuls + DVE evict (+DMA when
            # both halves staged)
            if nn == 0:
                ostg_t[sc] = p_ostg.tile([128, D], BF16, tag="ostg", name=f"ostg{sc}")
            stg = ostg_t[sc]
            ps = ps_pj.tile([128, 512], F32, tag="proj")
            for kc in range(2):
                nc.tensor.matmul(
                    ps[:],
                    mg_t[kc][:, sc * 128:(sc + 1) * 128],
                    wo_t[kc][:, nn * 512:(nn + 1) * 512],
                    start=(kc == 0), stop=(kc == 1))
            nc.vector.tensor_copy(stg[:, nn * 512:(nn + 1) * 512], ps[:])
            if nn == 1:
                nc.sync.dma_start(out_d[sc * 128:(sc + 1) * 128, :], stg[:])

        def oproj_quant(sc):
            oproj_half(sc, 0)
            oproj_half(sc, 1)

        mg_t = [p_mg.tile([128, S], MGDT, tag="mgT", name=f"mg{i}")
                for i in range(M // 128)]

        # ---- attention by head pair, scoresT orientation ----
        last_exp = [None]

        def _last_inst():
            return nc.inst_map[next(reversed(nc.inst_map))]

        def attend_pair(t, ic, drain):
            # heads (2t, 2t+1): head A on partitions 0:64 of q_t[t]/k_t[t],
            # head B on 64:128. Score matmuls for A and B are row-group
            # packed (tile_position (0,0) / (64,0)) and run concurrently.
            hA, hB = 2 * t, 2 * t + 1
            njc = 4 * (ic + 1)
            at_A = ps_at.tile([DH + 1, IC], F32, tag="at", name=f"atA{t}_{ic}")
            at_B = ps_at.tile([DH + 1, IC], F32, tag="at", name=f"atB{t}_{ic}")
            for p in range(0, njc, 2):
                psA = ps_sc.tile([128, 2 * IC], F32, tag="sc")
                psB = ps_sc.tile([128, 2 * IC], F32, tag="sc")
                prA = p_probs.tile([JC, 2 * IC], ATDT, tag="probs")
                prB = p_probs.tile([JC, 2 * IC], ATDT, tag="probs")
                offs = []
                for u in range(2):
                    jc = p + u
                    d = jc * JC - ic * IC
                    off = d if d >= 0 else 0   # diagonal: live cols [off, IC)
                    offs.append(off)
                    jsl = slice(jc * JC, (jc + 1) * JC)
                    isl = slice(ic * IC + off, (ic + 1) * IC)
                    nc.tensor.matmul(
                        psA[:, u * IC + off:(u + 1) * IC],
                        k_t[t][0:DH, jsl], q_t[t][0:DH, isl],
                        start=True, stop=True)
                    mmA = _last_inst()
                    if PAIR_ALIGN and u == 0 and last_exp[0] is not None:
                        # align the pair: A must not issue before BOTH psum
                        # tiles are free, else B trails A by a full exp and
                        # the row-group concurrency is lost
                        tile.add_dep_helper(mmA, last_exp[0], sync=True,
                                            reason="pair-align")
                    nc.tensor.matmul(
                        psB[:, u * IC + off:(u + 1) * IC],
                        k_t[t][DH:128, jsl], q_t[t][DH:128, isl],
                        start=True, stop=True)
                for u in range(2):
                    jc = p + u
                    d = jc * JC - ic * IC
                    if d >= 0:  # diagonal: additive -inf-ish mask pre-exp
                        msl = slice(u * IC + d, u * IC + d + JC)
                        nc.vector.tensor_add(psA[:, msl], psA[:, msl], tri_t[:])
                        nc.vector.tensor_add(psB[:, msl], psB[:, msl], tri_t[:])
                if offs[1] > 0:
                    # diagonal pair: exp only the live regions (never read
                    # uninitialized PSUM)
                    s0 = slice(offs[0], IC)
                    s1 = slice(IC + offs[1], 2 * IC)
                    nc.scalar.activation(prA[:, s0], psA[:, s0], EXP, scale=SCALE)
                    nc.scalar.activation(prA[:, s1], psA[:, s1], EXP, scale=SCALE)
                    nc.scalar.activation(prB[:, s0], psB[:, s0], EXP, scale=SCALE)
                    nc.scalar.activation(prB[:, s1], psB[:, s1], EXP, scale=SCALE)
                else:
                    nc.scalar.activation(prA[:], psA[:], EXP, scale=SCALE)
                    nc.scalar.activation(prB[:], psB[:], EXP, scale=SCALE)
                last_exp[0] = _last_inst()
                drain()
                for u in range(2):
                    jc = p + u
                    off = offs[u]
                    st, sp = (jc == 0), (jc == njc - 1)
                    nc.tensor.matmul(
                        at_A[:, off:IC],
                        v_t[jc][:, hA * VW:hA * VW + DH + 1],
                        prA[:, u * IC + off:(u + 1) * IC],
                        start=st, stop=sp)
                    nc.tensor.matmul(
                        at_B[:, off:IC],
                        v_t[jc][:, hB * VW:hB * VW + DH + 1],
                        prB[:, u * IC + off:(u + 1) * IC],
                        start=st, stop=sp)
            normalize_pair(t, ic, at_A, at_B)

        def normalize_pair(t, ic, at_A, at_B):
            # dens (row 64) -> f32r rows 0/1; one K=2 selector matmul
            # broadcasts den_A to partitions 0:64 and den_B to 64:128; one
            # [128,512] reciprocal; two multiplies into mgT.
            nc.vector.tensor_copy(denr_t[0:1, :], at_A[DH:DH + 1, :])
            nc.vector.tensor_copy(denr_t[32:33, :], at_B[DH:DH + 1, :])
            bc_ps = ps_pj.tile([128, IC], F32, tag="proj")
            nc.tensor.matmul(bc_ps[:], sel_t[:], denr_t[:], start=True, stop=True)
            bc_sb = p_small.tile([128, IC], F32, tag="bcast")
            nc.vector.reciprocal_approx_fast(bc_sb[:], bc_ps[:])
            isl = slice(ic * IC, (ic + 1) * IC)
            nc.vector.tensor_mul(mg_t[t][0:DH, isl], at_A[0:DH, :], bc_sb[0:DH, :])
            nc.vector.tensor_mul(mg_t[t][DH:128, isl], at_B[0:DH, :], bc_sb[DH:128, :])

        # ---- schedule ----
        # Pre-phase: only the s4=0 column block of qk tile 0 plus v chunks
        # 0..3 gate the first attention wave.
        qk_quant(0, 0, q_t, "qT", 0)
        qk_quant(0, M, k_t, "kT", 0)
        for sc in range(S // JC):
            v_premake(sc)
        for sc in range(4):
            v_quant(sc)

        # Keyed quanta queue, s4-major so each attention wave's inputs
        # complete just ahead of need; drained inside attention j-loops.
        Q = deque()

        def drain_n(n):
            for _ in range(n):
                if Q:
                    Q.popleft()[1]()

        def make_drain(n_per_iter):
            return lambda: drain_n(n_per_iter)

        def flush_through(key):
            while Q:
                k, fn = Q.popleft()
                fn()
                if k == key:
                    return

        Q.append(("q1s0", lambda: qk_quant(1, 0, q_t, "qT", 0)))
        Q.append(("k1s0", lambda: qk_quant(1, M, k_t, "kT", 0)))
        for s4 in range(1, 4):
            Q.append((f"q0s{s4}", lambda s4=s4: qk_quant(0, 0, q_t, "qT", s4)))
            Q.append((f"k0s{s4}", lambda s4=s4: qk_quant(0, M, k_t, "kT", s4)))
            Q.append((f"q1s{s4}", lambda s4=s4: qk_quant(1, 0, q_t, "qT", s4)))
            Q.append((f"k1s{s4}", lambda s4=s4: qk_quant(1, M, k_t, "kT", s4)))
            for sc in range(4 * s4, 4 * s4 + 4):
                Q.append((f"v{sc}", lambda sc=sc: v_quant(sc)))

        attend_pair(0, 0, make_drain(2))
        flush_through("k1s0")
        attend_pair(1, 0, make_drain(2))
        for sc in range(4):
            for nn in range(2):
                Q.append((f"o{sc}_{nn}", lambda sc=sc, nn=nn: oproj_half(sc, nn)))
        flush_through("v7")
        attend_pair(0, 1, make_drain(2))
        attend_pair(1, 1, make_drain(2))
        for sc in range(4, 8):
            for nn in range(2):
                Q.append((f"o{sc}_{nn}", lambda sc=sc, nn=nn: oproj_half(sc, nn)))
        flush_through("v11")
        attend_pair(0, 2, make_drain(1))
        attend_pair(1, 2, make_drain(1))
        for sc in range(8, 12):
            for nn in range(2):
                Q.append((f"o{sc}_{nn}", lambda sc=sc, nn=nn: oproj_half(sc, nn)))
        flush_through("v15")
        attend_pair(0, 3, make_drain(1))
        attend_pair(1, 3, make_drain(1))
        while Q:
            Q.popleft()[1]()
        for sc in range(12, 16):
            oproj_quant(sc)
        if dbg:
            if "x0" in dbg:
                nc.sync.dma_start(dbg["x0"][:], x_t[0][:].bitcast(F32))
                nc.sync.dma_start(dbg["w0"][:], w_t[0][:].bitcast(F32))
            nc.sync.dma_start(dbg["q0"][:], q_t[0][:])
            nc.sync.dma_start(dbg["k0"][:], k_t[0][:])
            nc.sync.dma_start(dbg["v0"][:], v_t[0][:])
            nc.sync.dma_start(dbg["mg0"][:], mg_t[0][:])


_NC_CACHE = None


def _get_nc():
    global _NC_CACHE
    if _NC_CACHE is None:
        _NC_CACHE = _build_nc()
    return _NC_CACHE


_np_xw = ml_dtypes.bfloat16 if XW16 else np.float32
_np_ld = ml_dtypes.bfloat16 if LOADBF else np.float32


def _ld_pack(a):
    # bf16 bytes shipped through an f32-typed DMA (bf16-typed DMAs race)
    if LOADBF:
        return np.ascontiguousarray(a.astype(ml_dtypes.bfloat16)).view(np.float32)
    return a
_np_at = ml_dtypes.bfloat16 if AT16 else np.float32
_np_mg = ml_dtypes.bfloat16 if MG16 else np.float32


def _sel_mat():
    # sel[k, m]: broadcast-selector for the pair normalize matmul
    # (den_A lives on partition 0, den_B on partition 32 of the den tile)
    s = np.zeros((33, 128), np.float32)
    s[0, 0:64] = 1.0
    s[32, 64:128] = 1.0
    return s


def _tri_mask():
    # additive mask: 0 where kept (c >= r), -704 where dropped (exp -> 0)
    r = np.arange(JC)[:, None]
    c = np.arange(JC)[None, :]
    return np.where(c >= r, 0.0, -704.0).astype(np.float32)


def _prepare_in_maps(inputs):
    x = np.asarray(inputs["in_features"], dtype=np.float32)
    wqT = np.ascontiguousarray(np.asarray(inputs["q_proj_weight"], np.float32).T)
    wkT = np.ascontiguousarray(np.asarray(inputs["k_proj_weight"], np.float32).T)
    wvT = np.ascontiguousarray(np.asarray(inputs["v_proj_weight"], np.float32).T)
    woT = np.ascontiguousarray(np.asarray(inputs["o_proj_weight"], np.float32).T)
    xT = [np.ascontiguousarray(x[b].T) for b in range(B)]
    tri = _tri_mask()

    in_maps = []
    for c in range(NCORES):
        b, g = divmod(c, HPG)
        ms = slice(g * M, (g + 1) * M)
        in_maps.append({
            "xT": xT[b],
            "wqkvT": np.ascontiguousarray(
                np.concatenate([wqT[:, ms], wkT[:, ms], wvT[:, ms]], axis=1)),
            "woT": np.ascontiguousarray(woT[ms, :]),
            "tri": tri,
            "sel": _sel_mat(),
            "ones_b": np.ones((JC, HPG), np.float32),
        })
    return in_maps


def kernel(q_proj_weight, k_proj_weight, v_proj_weight, o_proj_weight, in_features):
    in_dtype = np.asarray(in_features).dtype
    in_maps = _prepare_in_maps({
        "q_proj_weight": q_proj_weight,
        "k_proj_weight": k_proj_weight,
        "v_proj_weight": v_proj_weight,
        "o_proj_weight": o_proj_weight,
        "in_features": in_features,
    })
    nc = _get_nc()
    res = bass_utils.run_bass_kernel_spmd(nc, in_maps, core_ids=list(range(NCORES)))
    out = np.zeros((B, S, D), dtype=np.float32)
    for c in range(NCORES):
        out[c // HPG] += np.asarray(res.results[c]["out"], dtype=np.float32)
    return out.astype(in_dtype)


# revision 46
# speedup vs baseline: 1.0588x; 1.0588x over previous
"""Multi-head self-attention (B=2, S=2048, D=1024, H=16, causal) on 8 NeuronCores.

Sharding: core c = 4*b + g handles batch b and heads 4g..4g+3 (batch x
head-group parallel). Per core:
  - q/k projections in transposed layout qT/kT [dh, s] (dh on partitions,
    two heads stacked per 128-partition tile), evicted to bf16
  - v projection in natural layout [s, dh], bf16, with a fused ones-column
    per head (softmax denominator comes free during the AV matmul); tiles
    are pre-memset to 1.0 so the ones columns need no strided write
  - attention per HEAD PAIR in scoresT [j, i] orientation: bf16 score
    matmuls for the two heads at row groups 0:64 / 64:128, causal masking
    as an additive -704 bias on the PSUM scores BEFORE the exp (so exp
    underflows to zero; avoids in-place bf16 multiplies, which corrupt),
    exp sliced to live regions on diagonal chunks, bf16 probs, AV matmuls
    column-sliced to the causal live region
  - softmax normalization per pair: dens to f32r rows 0/32 of a shared
    tile, one K=33 selector matmul broadcasts both dens across partitions,
    one [128,512] reciprocal, two multiplies into mergedT
  - o-projection partials staged via SBUF and DMA'd out; all projection
    work is sliced into ~0.5-2us quanta drained inside the attention
    j-loop to fill the PE's exp-wait gaps
Host sums the 4 partial outputs per batch (the only cross-core reduction).

Projections run in f32r (weights with 128 columns in bf16 would take the
compiler's FWL path, which corrupts weight loads in this kernel); the
attention matmuls (K=64 scores / M=65 AV, both FWL-ineligible) run in bf16.
"""

import numpy as np
import ml_dtypes
from collections import deque

import concourse.bass as bass
from concourse import bacc
import concourse.mybir as mybir
import concourse.tile as tile
from concourse import bass_utils

F32 = mybir.dt.float32
F32R = mybir.dt.float32r
BF16 = mybir.dt.bfloat16
EXP = mybir.ActivationFunctionType.Exp

B, S, D = 2, 2048, 1024
H, DH = 16, 64
NCORES = 8
HPG = 4                  # heads per group (per core)
M = HPG * DH             # 256 per-core head dims
VW = DH + 2              # v tile cols per head: 64 v + 1 ones + 1 pad (even stride)
DC = D // 128            # 8 contraction chunks for projections
IC = 512                 # i (query) chunk for attention
JC = 128                 # j (key) chunk for attention
SCALE = 1.0 / np.sqrt(DH)
import os
PAIR_ALIGN = os.environ.get("PAIR_ALIGN", "0") == "1"
LOADBF = os.environ.get("LOADBF", "0") == "1"
XW16 = os.environ.get("XW16", "0") == "1"   # x/w/wo loads bf16
QK16 = os.environ.get("QK16", "1") == "1"   # q/k tiles bf16
AT16 = os.environ.get("AT16", "1") == "1"   # v/probs/tri bf16
MG16 = os.environ.get("MG16", "0") == "1"   # mg bf16 (wo must match)
XWDT = BF16 if XW16 else F32R
QKDT = BF16 if QK16 else F32R
ATDT = BF16 if AT16 else F32R
ATMT = BF16 if AT16 else F32
MGDT = BF16 if MG16 else F32R


def _build_nc():
    nc = bacc.Bacc("TRN2", target_bir_lowering=False, debug=False)

    xT_d = nc.dram_tensor("xT", [D, S // 2] if LOADBF else [D, S], F32R, kind="ExternalInput").ap()
    wqkv_d = nc.dram_tensor("wqkvT", [D, 3 * M // 2] if LOADBF else [D, 3 * M], F32R, kind="ExternalInput").ap()
    woT_d = nc.dram_tensor("woT", [M, D], MGDT, kind="ExternalInput").ap()
    tri_d = nc.dram_tensor("tri", [JC, JC], F32, kind="ExternalInput").ap()
    sel_d = nc.dram_tensor("sel", [33, 128], F32R, kind="ExternalInput").ap()
    onesb_d = nc.dram_tensor("ones_b", [JC, HPG], ATDT, kind="ExternalInput").ap()
    out_d = nc.dram_tensor("out", [S, D], BF16, kind="ExternalOutput").ap()
    dbg = {}
    if os.environ.get("KDBG") == "1":
        dbg["q0"] = nc.dram_tensor("dbg_q0", [128, S], QKDT, kind="ExternalOutput").ap()
        dbg["k0"] = nc.dram_tensor("dbg_k0", [128, S], QKDT, kind="ExternalOutput").ap()
        dbg["v0"] = nc.dram_tensor("dbg_v0", [JC, HPG * VW], ATDT, kind="ExternalOutput").ap()
        dbg["mg0"] = nc.dram_tensor("dbg_mg0", [128, S], MGDT, kind="ExternalOutput").ap()
        dbg["pr0"] = nc.dram_tensor("dbg_pr0", [JC, IC], ATDT, kind="ExternalOutput").ap()
        dbg["x0"] = nc.dram_tensor("dbg_x0", [128, S], F32, kind="ExternalOutput").ap()
        dbg["w0"] = nc.dram_tensor("dbg_w0", [128, 3 * M], F32, kind="ExternalOutput").ap()

    with tile.TileContext(nc) as tc:
        _body(tc, xT_d, wqkv_d, woT_d, tri_d, sel_d, onesb_d, out_d, dbg)
    nc.compile()
    return nc


def _body(tc, xT_d, wqkv_d, woT_d, tri_d, sel_d, onesb_d, out_d, dbg=None):
    nc = tc.nc
    from contextlib import ExitStack
    ctx = ExitStack()
    with ctx:
        p_x = ctx.enter_context(tc.tile_pool(name="x", bufs=DC))
        p_w = ctx.enter_context(tc.tile_pool(name="w", bufs=DC))
        p_xl = ctx.enter_context(tc.tile_pool(name="xl", bufs=DC))
        p_wl = ctx.enter_context(tc.tile_pool(name="wl", bufs=DC))
        p_wo = ctx.enter_context(tc.tile_pool(name="wo", bufs=2))
        p_qk = ctx.enter_context(tc.tile_pool(name="qk", bufs=2))
        p_v = ctx.enter_context(tc.tile_pool(name="v", bufs=S // JC))
        p_mg = ctx.enter_context(tc.tile_pool(name="mg", bufs=2))
        p_probs = ctx.enter_context(tc.tile_pool(name="probs", bufs=6))
        p_small = ctx.enter_context(tc.tile_pool(name="small", bufs=2))
        p_mask = ctx.enter_context(tc.tile_pool(name="mask", bufs=1))
        p_ostg = ctx.enter_context(tc.tile_pool(name="ostg", bufs=2))
        p_ones = ctx.enter_context(tc.tile_pool(name="ones", bufs=1))

        # PSUM: 8 banks total.
        ps_sc = ctx.enter_context(tc.tile_pool(name="pss", bufs=2, space="PSUM"))  # 2x[128,1024] = 4 banks
        ps_at = ctx.enter_context(tc.tile_pool(name="psa", bufs=2, space="PSUM"))  # 2x[65,512]  = 2 banks
        ps_pj = ctx.enter_context(tc.tile_pool(name="psp", bufs=2, space="PSUM"))  # 2x[128,512] = 2 banks

        # ---- HAM pre-warm: keep the PE activity monitor busy while the
        # first x/w tiles stream in so the clock gate is at full rate when
        # the real projections start.
        wrm = p_ones.tile([128, 512], F32, tag="warm")
        nc.vector.memset(wrm[:], 1.0)
        wrm_ps = ps_pj.tile([128, 512], F32, tag="proj", name="warmps")
        for r in range(9):
            nc.tensor.matmul(wrm_ps[:], wrm[:, 0:128], wrm[:],
                             start=(r == 0), stop=(r == 8))
        nc.scalar.copy(wrm[:, 0:1], wrm_ps[:, 0:1])  # keep alive vs DCE

        # ---- input loads, interleaved in consumption order. x/w stream in
        # as bf16 (halves the DMA-bound ramp) and are converted to f32r on
        # DVE while the ramp is otherwise idle (bf16 matmul weights with 128
        # cols trip the FWL path, which corrupts in this kernel - so the
        # projections consume f32r).
        w_t, x_t = [], []
        for dc in range(DC):
            if LOADBF:
                wl = p_wl.tile([128, 3 * M // 2], F32R, tag="wl")
                nc.sync.dma_start(wl[:], wqkv_d[dc * 128:(dc + 1) * 128, :])
                xl = p_xl.tile([128, S // 2], F32R, tag="xl")
                nc.sync.dma_start(xl[:], xT_d[dc * 128:(dc + 1) * 128, :])
                wt = p_w.tile([128, 3 * M], F32R, tag="w")
                nc.scalar.copy(wt[:], wl[:].bitcast(BF16))
                w_t.append(wt)
                xt = p_x.tile([128, S], F32R, tag="x")
                nc.scalar.copy(xt[:], xl[:].bitcast(BF16))
                x_t.append(xt)
            else:
                wt = p_w.tile([128, 3 * M], F32R, tag="w")
                nc.sync.dma_start(wt[:], wqkv_d[dc * 128:(dc + 1) * 128, :])
                w_t.append(wt)
                xt = p_x.tile([128, S], F32R, tag="x")
                nc.sync.dma_start(xt[:, 0:1024], xT_d[dc * 128:(dc + 1) * 128, 0:1024])
                x_t.append(xt)
        if not LOADBF:
            # second x column half lands after the first, so the s4<=1
            # projection chains can start at ~half the input-load time
            for dc in range(DC):
                nc.sync.dma_start(x_t[dc][:, 1024:2048],
                                  xT_d[dc * 128:(dc + 1) * 128, 1024:2048])
        wo_t = []
        for kc in range(2):
            t = p_wo.tile([128, D], MGDT, tag="wo")
            nc.sync.dma_start(t[:], woT_d[kc * 128:(kc + 1) * 128, :])
            wo_t.append(t)
        tri_t = p_mask.tile([JC, JC], F32, tag="tri")
        nc.sync.dma_start(tri_t[:], tri_d[:])
        sel_t = p_ones.tile([33, 128], F32R, tag="sel")
        nc.sync.dma_start(sel_t[:], sel_d[:])
        denr_t = p_ones.tile([33, IC], F32R, tag="denr")
        nc.vector.memset(denr_t[:].bitcast(F32), 0.0)
        onesb_t = p_ones.tile([JC, HPG], ATDT, tag="onesb")
        nc.sync.dma_start(onesb_t[:], onesb_d[:])

        # ---- projection building blocks ----
        q_t, k_t = {}, {}

        def qk_quant(mc, woff, store, tg, s4):
            # one qT/kT [128, 512] block: 8-matmul contraction chain + one
            # DVE eviction copy
            sl = slice(s4 * 512, (s4 + 1) * 512)
            dst = store.get(mc)
            if dst is None:
                dst = p_qk.tile([128, S], QKDT, tag=tg, name=f"{tg}{mc}")
                store[mc] = dst
            ps = ps_pj.tile([128, 512], F32, tag="proj")
            for dc in range(DC):
                nc.tensor.matmul(
                    ps[:],
                    w_t[dc][:, woff + mc * 128:woff + (mc + 1) * 128],
                    x_t[dc][:, sl],
                    start=(dc == 0), stop=(dc == DC - 1))
            nc.vector.tensor_copy(dst[:, sl], ps[:])

        v_pre = {}

        def v_premake(sc):
            # pre-created + memset to 1.0 (ones cols come for free; the
            # eviction overwrites the value cols)
            vt = p_v.tile([JC, HPG * VW], ATDT, tag="v", name=f"v{sc}")
            if AT16:
                nc.vector.memset(vt[:].bitcast(mybir.dt.uint16), 16256)  # bf16 1.0
            else:
                nc.vector.memset(vt[:].bitcast(F32), 1.0)
            v_pre[sc] = vt

        def v_quant(sc):
            # v[s, m] tile for j-chunk sc: per head h cols h*66..h*66+63 = v,
            # col h*66+64 = 1.0 (softmax denominator column)
            vt = v_pre[sc]
            ps = ps_pj.tile([128, 512], F32, tag="proj")
            for dc in range(DC):
                nc.tensor.matmul(
                    ps[:, 0:M],
                    x_t[dc][:, sc * 128:(sc + 1) * 128],
                    w_t[dc][:, 2 * M:3 * M],
                    start=(dc == 0), stop=(dc == DC - 1))
            nc.vector.tensor_copy(
                vt[:].rearrange("p (h e) -> p h e", h=HPG)[:, :, 0:DH],
                ps[:, 0:M].rearrange("p (h d) -> p h d", h=HPG))
            v_t[sc] = vt

        v_t = {}

        ostg_t = {}

        def oproj_half(sc, nn):
            # half of an out[s, o] partial: 2 matmuls + DVE evict (+DMA when
            # both halves staged)
            if nn == 0:
                ostg_t[sc] = p_ostg.tile([128, D], BF16, tag="ostg", name=f"ostg{sc}")
            stg = ostg_t[sc]
            ps = ps_pj.tile([128, 512], F32, tag="proj")
            for kc in range(2):
                nc.tensor.matmul(
                    ps[:],
                    mg_t[kc][:, sc * 128:(sc + 1) * 128],
                    wo_t[kc][:, nn * 512:(nn + 1) * 512],
                    start=(kc == 0), stop=(kc == 1))
            nc.vector.tensor_copy(stg[:, nn * 512:(nn + 1) * 512], ps[:])
            if nn == 1:
                nc.sync.dma_start(out_d[sc * 128:(sc + 1) * 128, :], stg[:])

        def oproj_quant(sc):
            oproj_half(sc, 0)
            oproj_half(sc, 1)

        mg_t = [p_mg.tile([128, S], MGDT, tag="mgT", name=f"mg{i}")
                for i in range(M // 128)]

        # ---- attention by head pair, scoresT orientation ----
        last_exp = [None]

        def _last_inst():
            return nc.inst_map[next(reversed(nc.inst_map))]

        def attend_pair(t, ic, drain):
            # heads (2t, 2t+1): head A on partitions 0:64 of q_t[t]/k_t[t],
            # head B on 64:128. Score matmuls for A and B are row-group
            # packed (tile_position (0,0) / (64,0)) and run concurrently.
            hA, hB = 2 * t, 2 * t + 1
            njc = 4 * (ic + 1)
            at_A = ps_at.tile([DH + 1, IC], F32, tag="at", name=f"atA{t}_{ic}")
            at_B = ps_at.tile([DH + 1, IC], F32, tag="at", name=f"atB{t}_{ic}")
            for p in range(0, njc, 2):
                psA = ps_sc.tile([128, 2 * IC], F32, tag="sc")
                psB = ps_sc.tile([128, 2 * IC], F32, tag="sc")
                prA = p_probs.tile([JC, 2 * IC], ATDT, tag="probs")
                prB = p_probs.tile([JC, 2 * IC], ATDT, tag="probs")
                offs = []
                for u in range(2):
                    jc = p + u
                    d = jc * JC - ic * IC
                    off = d if d >= 0 else 0   # diagonal: live cols [off, IC)
                    offs.append(off)
                    jsl = slice(jc * JC, (jc + 1) * JC)
                    isl = slice(ic * IC + off, (ic + 1) * IC)
                    nc.tensor.matmul(
                        psA[:, u * IC + off:(u + 1) * IC],
                        k_t[t][0:DH, jsl], q_t[t][0:DH, isl],
                        start=True, stop=True)
                    mmA = _last_inst()
                    if PAIR_ALIGN and u == 0 and last_exp[0] is not None:
                        # align the pair: A must not issue before BOTH psum
                        # tiles are free, else B trails A by a full exp and
                        # the row-group concurrency is lost
                        tile.add_dep_helper(mmA, last_exp[0], sync=True,
                                            reason="pair-align")
                    nc.tensor.matmul(
                        psB[:, u * IC + off:(u + 1) * IC],
                        k_t[t][DH:128, jsl], q_t[t][DH:128, isl],
                        start=True, stop=True)
                for u in range(2):
                    jc = p + u
                    d = jc * JC - ic * IC
                    if d >= 0:  # diagonal: additive -inf-ish mask pre-exp
                        msl = slice(u * IC + d, u * IC + d + JC)
                        nc.vector.tensor_add(psA[:, msl], psA[:, msl], tri_t[:])
                        nc.vector.tensor_add(psB[:, msl], psB[:, msl], tri_t[:])
                if offs[1] > 0:
                    # diagonal pair: exp only the live regions (never read
                    # uninitialized PSUM)
                    s0 = slice(offs[0], IC)
                    s1 = slice(IC + offs[1], 2 * IC)
                    nc.scalar.activation(prA[:, s0], psA[:, s0], EXP, scale=SCALE)
                    nc.scalar.activation(prA[:, s1], psA[:, s1], EXP, scale=SCALE)
                    nc.scalar.activation(prB[:, s0], psB[:, s0], EXP, scale=SCALE)
                    nc.scalar.activation(prB[:, s1], psB[:, s1], EXP, scale=SCALE)
                else:
                    nc.scalar.activation(prA[:], psA[:], EXP, scale=SCALE)
                    nc.scalar.activation(prB[:], psB[:], EXP, scale=SCALE)
                last_exp[0] = _last_inst()
                drain()
                for u in range(2):
                    jc = p + u
                    off = offs[u]
                    st, sp = (jc == 0), (jc == njc - 1)
                    nc.tensor.matmul(
                        at_A[:, off:IC],
                        v_t[jc][:, hA * VW:hA * VW + DH + 1],
                        prA[:, u * IC + off:(u + 1) * IC],
                        start=st, stop=sp)
                    nc.tensor.matmul(
                        at_B[:, off:IC],
                        v_t[jc][:, hB * VW:hB * VW + DH + 1],
                        prB[:, u * IC + off:(u + 1) * IC],
                        start=st, stop=sp)
            normalize_pair(t, ic, at_A, at_B)

        def normalize_pair(t, ic, at_A, at_B):
            # dens (row 64) -> f32r rows 0/1; one K=2 selector matmul
            # broadcasts den_A to partitions 0:64 and den_B to 64:128; one
            # [128,512] reciprocal; two multiplies into mgT.
            nc.vector.tensor_copy(denr_t[0:1, :], at_A[DH:DH + 1, :])
            nc.vector.tensor_copy(denr_t[32:33, :], at_B[DH:DH + 1, :])
            bc_ps = ps_pj.tile([128, IC], F32, tag="proj")
            nc.tensor.matmul(bc_ps[:], sel_t[:], denr_t[:], start=True, stop=True)
            bc_sb = p_small.tile([128, IC], F32, tag="bcast")
            nc.vector.reciprocal_approx_fast(bc_sb[:], bc_ps[:])
            isl = slice(ic * IC, (ic + 1) * IC)
            nc.vector.tensor_mul(mg_t[t][0:DH, isl], at_A[0:DH, :], bc_sb[0:DH, :])
            nc.vector.tensor_mul(mg_t[t][DH:128, isl], at_B[0:DH, :], bc_sb[DH:128, :])

        # ---- schedule ----
        # Pre-phase (DMA-bound window): qk tile 0 + v chunks 0..3.
        for woff, store, tg in ((0, q_t, "qT"), (M, k_t, "kT")):
            for s4 in range(4):
                qk_quant(0, woff, store, tg, s4)
        for sc in range(S // JC):
            v_premake(sc)
        for sc in range(4):
            v_quant(sc)

        # Quanta queue: projection work drained inside attention j-loops to
        # fill the PE's exp-wait gaps.
        Q = deque()

        def make_drain(n_per_iter):
            def drain():
                for _ in range(n_per_iter):
                    if Q:
                        Q.popleft()()
            return drain

        def flush(leave=0):
            while len(Q) > leave:
                Q.popleft()()

        for woff, store, tg in ((0, q_t, "qT"), (M, k_t, "kT")):
            for s4 in range(4):
                Q.append(lambda s4=s4, woff=woff, store=store, tg=tg:
                         qk_quant(1, woff, store, tg, s4))
        for sc in range(4, 16):
            Q.append(lambda sc=sc: v_quant(sc))

        attend_pair(0, 0, make_drain(2))   # drains qk1 quanta
        flush(leave=12)                    # qk1 must finish before pair t=1
        attend_pair(1, 0, make_drain(2))
        flush(leave=8)                     # v4..7 before ic=1
        for sc in range(4):
            for nn in range(2):
                Q.append(lambda sc=sc, nn=nn: oproj_half(sc, nn))
        attend_pair(0, 1, make_drain(2))
        attend_pair(1, 1, make_drain(2))
        flush(leave=4)                     # v8..11 before ic=2
        for sc in range(4, 8):
            for nn in range(2):
                Q.append(lambda sc=sc, nn=nn: oproj_half(sc, nn))
        attend_pair(0, 2, make_drain(1))
        attend_pair(1, 2, make_drain(1))
        flush()                            # v12..15 before ic=3
        for sc in range(8, 12):
            for nn in range(2):
                Q.append(lambda sc=sc, nn=nn: oproj_half(sc, nn))
        attend_pair(0, 3, make_drain(1))
        attend_pair(1, 3, make_drain(1))
        flush()
        for sc in range(12, 16):
            oproj_quant(sc)
        if dbg:
            if "x0" in dbg:
                nc.sync.dma_start(dbg["x0"][:], x_t[0][:].bitcast(F32))
                nc.sync.dma_start(dbg["w0"][:], w_t[0][:].bitcast(F32))
            nc.sync.dma_start(dbg["q0"][:], q_t[0][:])
            nc.sync.dma_start(dbg["k0"][:], k_t[0][:])
            nc.sync.dma_start(dbg["v0"][:], v_t[0][:])
            nc.sync.dma_start(dbg["mg0"][:], mg_t[0][:])


_NC_CACHE = None


def _get_nc():
    global _NC_CACHE
    if _NC_CACHE is None:
        _NC_CACHE = _build_nc()
    return _NC_CACHE


_np_xw = ml_dtypes.bfloat16 if XW16 else np.float32
_np_ld = ml_dtypes.bfloat16 if LOADBF else np.float32


def _ld_pack(a):
    # bf16 bytes shipped through an f32-typed DMA (bf16-typed DMAs race)
    if LOADBF:
        return np.ascontiguousarray(a.astype(ml_dtypes.bfloat16)).view(np.float32)
    return a
_np_at = ml_dtypes.bfloat16 if AT16 else np.float32
_np_mg = ml_dtypes.bfloat16 if MG16 else np.float32


def _sel_mat():
    # sel[k, m]: broadcast-selector for the pair normalize matmul
    # (den_A lives on partition 0, den_B on partition 32 of the den tile)
    s = np.zeros((33, 128), np.float32)
    s[0, 0:64] = 1.0
    s[32, 64:128] = 1.0
    return s


def _tri_mask():
    # additive mask: 0 where kept (c >= r), -704 where dropped (exp -> 0)
    r = np.arange(JC)[:, None]
    c = np.arange(JC)[None, :]
    return np.where(c >= r, 0.0, -704.0).astype(np.float32)


def _prepare_in_maps(inputs):
    x = np.asarray(inputs["in_features"], dtype=np.float32)
    wqT = np.ascontiguousarray(np.asarray(inputs["q_proj_weight"], np.float32).T)
    wkT = np.ascontiguousarray(np.asarray(inputs["k_proj_weight"], np.float32).T)
    wvT = np.ascontiguousarray(np.asarray(inputs["v_proj_weight"], np.float32).T)
    woT = np.ascontiguousarray(np.asarray(inputs["o_proj_weight"], np.float32).T)
    xT = [np.ascontiguousarray(x[b].T) for b in range(B)]
    tri = _tri_mask()

    in_maps = []
    for c in range(NCORES):
        b, g = divmod(c, HPG)
        ms = slice(g * M, (g + 1) * M)
        in_maps.append({
            "xT": xT[b],
            "wqkvT": np.ascontiguousarray(
                np.concatenate([wqT[:, ms], wkT[:, ms], wvT[:, ms]], axis=1)),
            "woT": np.ascontiguousarray(woT[ms, :]),
            "tri": tri,
            "sel": _sel_mat(),
            "ones_b": np.ones((JC, HPG), np.float32),
        })
    return in_maps


def kernel(q_proj_weight, k_proj_weight, v_proj_weight, o_proj_weight, in_features):
    in_dtype = np.asarray(in_features).dtype
    in_maps = _prepare_in_maps({
        "q_proj_weight": q_proj_weight,
        "k_proj_weight": k_proj_weight,
        "v_proj_weight": v_proj_weight,
        "o_proj_weight": o_proj_weight,
        "in_features": in_features,
    })
    nc = _get_nc()
    res = bass_utils.run_bass_kernel_spmd(nc, in_maps, core_ids=list(range(NCORES)))
    out = np.zeros((B, S, D), dtype=np.float32)
    for c in range(NCORES):
        out[c // HPG] += np.asarray(res.results[c]["out"], dtype=np.float32)
    return out.astype(in_dtype)
